# revision 26
# baseline (speedup 1.0000x reference)
"""Enformer-style relative-position attention (nn_Attention_27925877358942) for
8 Trainium2 NeuronCores.

Contract: kernel(**inputs) takes the FULL unsharded inputs (keys as in
setup_inputs()) and returns the full [1, 4096, 1536] float32 output.

Sharding: one head per core (8 heads / 8 cores). Host precomputes the
deterministic positional-feature table and x^T in fp16, slices per-head
weights, runs the SPMD Bass kernel via run_bass_kernel_spmd, and sums the
per-head output projections (+ b_out).

Device pipeline per core (head h), N=4096, d=64:
  - q^T,k^T (fp16, [64,N]) and v ([N,65] with ones col) projections on PE
  - r^T = (pos @ Wrelk_h)^T from the positional table
  - per query tile I: window logits em = exp((q+bp) . r[t0:t0+4223]) (ACT, bf16)
  - relative_shift via DRAM roundtrip: sheared strided read
      shr[di, j] = em[di, 127-di+j] (partition step = rowpitch-1 elements)
  - content logits transposed C^T = k_J . q_I (PE), exp on ACT
  - pT = exp(C^T) * transpose(shr) (PE transpose + DVE multiply, bf16)
  - O = pT.T @ [v|1] accumulated in PSUM; epilogue normalizes by the row sums
    and applies the per-head slice of W_out; host sums partials over heads.

This walrus build accepts at most ONE sync wait per instruction, so after
Tile scheduling every multi-wait instruction is split by inserting
wait-carrying NoOps just before it on the same engine (split_multi_waits),
and the Tile tail drain is built with the same constraint.
"""


_DRAIN_PATCHED = [False]


def _patch_tile_drain():
    if _DRAIN_PATCHED[0]:
        return
    _DRAIN_PATCHED[0] = True
    import concourse.tile as tile_mod
    from concourse.vector_clock import ScopedClock

    MAX_WAITS = 1

    def _drain_and_barrier(self, tick_clock, wait_clock):
        nc = self.nc
        drain_inst = nc.sync.drain()
        wait_clock.add_sem_waits(drain_inst.ins, ScopedClock({None: tick_clock.global_clock}))
        si = drain_inst.ins.sync_info
        waits = list(si.on_wait) if si is not None and si.on_wait else []
        if len(waits) > MAX_WAITS:
            si.on_wait = waits[:MAX_WAITS]
            rest = waits[MAX_WAITS:]
            import concourse.mybir as _mb
            for i in range(0, len(rest), MAX_WAITS):
                extra = nc.sync.drain()
                esi = extra.ins.sync_info
                if esi is None:
                    extra.ins.sync_info = _mb.SyncInfo(on_wait=rest[i:i + MAX_WAITS], on_update=[])
                else:
                    esi.on_wait = rest[i:i + MAX_WAITS]
        nc.all_engine_barrier()
        assert self.sems is not None
        popped = nc._tile_sem_poison_stack.pop()
        assert popped is self._sem_poison
        nc.clear_and_free_semaphores(list(self.sems.allocated().values()))
        nc.all_engine_barrier()

    tile_mod.TileContext._drain_and_barrier = _drain_and_barrier


def split_multi_waits(nc):
    """This walrus build allows at most ONE sync wait per instruction.
    Move extra waits onto InstNoOp carriers inserted just before, on the
    same engine queue (sequencers execute in order, so semantics hold)."""
    import concourse.mybir as mb
    n_split = 0
    for fn in nc.m.functions:
        for bb in fn.blocks:
            insts = list(bb.instructions)
            out = []
            for inst in insts:
                si = inst.sync_info
                waits = list(si.on_wait) if si is not None and si.on_wait else []
                if len(waits) > 1:
                    for w in waits[:-1]:
                        n_split += 1
                        nop = mb.InstNoOp(
                            name=f"waitsplit-{n_split}",
                            engine=inst.engine,
                            sync_info=mb.SyncInfo(on_wait=[w], on_update=[]),
                        )
                        out.append(nop)
                    si.on_wait = [waits[-1]]
                out.append(inst)
            if len(out) != len(insts):
                bb.instructions[:] = out
    return n_split


import math
from contextlib import ExitStack

import numpy as np

import concourse.bass as bass
import concourse.tile as tile
from concourse import mybir
from concourse.bass import ts, ds
from concourse.masks import make_identity

F32 = mybir.dt.float32
BF16 = mybir.dt.bfloat16
FP16 = mybir.dt.float16
U16 = mybir.dt.uint16
AF = mybir.ActivationFunctionType

DIM = 1536
H = 8
D = 64
NSEG = None  # set by build() from the static positional segmentation


def pos_segments(N):
    """Static segmentation of the 2N-1 relative positions: the central-mask
    features are piecewise constant in the distance, so the rel-k table has
    only ~157 distinct columns. Returns (seg_starts, seg_of)."""
    pos = get_positional_embed_np(N, 192)
    diffs = np.any(pos[1:] != pos[:-1], axis=1)
    seg_starts = np.concatenate([[0], np.nonzero(diffs)[0] + 1]).astype(np.int64)
    seg_of = np.zeros(2 * N - 1, np.int64)
    seg_of[seg_starts] = 1
    seg_of = np.cumsum(seg_of) - 1
    return seg_starts, seg_of


def build(N, split_waits=True, ic_chunk=1024):
    Q = N // 128           # query tiles
    NJ = N // 128          # key tiles
    WN = N + 128           # rel window width per q-tile (incl. 1 pad col)
    KD = DIM // 128        # contraction tiles for projections
    NC = N // 512          # projection chunks (one per xT DMA)
    S = len(pos_segments(N)[0])  # distinct rel-k columns (157 for N=4096)
    WCOLS = (2 * N - 1 + WN + 15) // 16  # wrapped master index cols

    nc = bass.Bass("TRN2", target_bir_lowering=False, debug=False)

    xT_d = nc.dram_tensor("xT", [DIM, N], FP16, kind="ExternalInput")
    pseg_d = nc.dram_tensor("pseg", [192, S], FP16, kind="ExternalInput")
    widx_d = nc.dram_tensor("widx", [128, WCOLS], U16, kind="ExternalInput")
    wq_d = nc.dram_tensor("wq", [DIM, D], FP16, kind="ExternalInput")
    wk_d = nc.dram_tensor("wk", [DIM, D], FP16, kind="ExternalInput")
    wv_d = nc.dram_tensor("wv", [DIM, D], FP16, kind="ExternalInput")
    wrk_d = nc.dram_tensor("wrk", [192, D], FP16, kind="ExternalInput")
    wo_d = nc.dram_tensor("wo", [D, DIM], BF16, kind="ExternalInput")
    bc_d = nc.dram_tensor("bc", [D, 1], F32, kind="ExternalInput")
    bp_d = nc.dram_tensor("bp", [D, 1], F32, kind="ExternalInput")
    out_d = nc.dram_tensor("out", [N, DIM], FP16, kind="ExternalOutput")
    em_d = nc.dram_tensor("em_scratch", [Q * 128, WN], BF16, kind="Internal")

    scale = D ** -0.5

    with tile.TileContext(nc) as tc, ExitStack() as ctx:
        consts = ctx.enter_context(tc.tile_pool(name="consts", bufs=1))
        persist = ctx.enter_context(tc.tile_pool(name="persist", bufs=1))
        work = ctx.enter_context(tc.tile_pool(name="work", bufs=2))
        ecpool = ctx.enter_context(tc.tile_pool(name="ecpool", bufs=2))
        upool = ctx.enter_context(tc.tile_pool(name="upool", bufs=3))
        empool = ctx.enter_context(tc.tile_pool(name="empool", bufs=4))
        shrpool = ctx.enter_context(tc.tile_pool(name="shrpool", bufs=5))
        sm = ctx.enter_context(tc.tile_pool(name="sm", bufs=2))
        ppool_m = ctx.enter_context(tc.tile_pool(name="ppool_m", bufs=1, space="PSUM"))
        ppool_ct = ctx.enter_context(tc.tile_pool(name="ppool_ct", bufs=2, space="PSUM"))
        ppool_st = ctx.enter_context(tc.tile_pool(name="ppool_st", bufs=2, space="PSUM"))
        ppool_epi = ctx.enter_context(tc.tile_pool(name="ppool_epi", bufs=1, space="PSUM"))
        ppool_op = ctx.enter_context(tc.tile_pool(name="ppool_op", bufs=2, space="PSUM"))

        # ---- constants (small loads via ACT queue; SP stays for x/em/shear) --
        ident = consts.tile([128, 128], BF16, tag="ident")
        make_identity(nc, ident[:])
        bc_sb = consts.tile([D, 1], F32, tag="bc")
        nc.scalar.dma_start(out=bc_sb[:], in_=bc_d.ap())
        bp_sb = consts.tile([D, 1], F32, tag="bp")
        nc.scalar.dma_start(out=bp_sb[:], in_=bp_d.ap())
        wo_sb = consts.tile([D, DIM], BF16, tag="wo")
        nc.scalar.dma_start(out=wo_sb[:], in_=wo_d.ap())
        wqk_sb = consts.tile([128, KD, 2 * D], FP16, tag="wqk")
        wv_sb = consts.tile([128, KD, D], FP16, tag="wv")
        nc.scalar.dma_start(out=wqk_sb[:, :, 0:D],
                            in_=wq_d.ap().rearrange("(t p) c -> p t c", p=128))
        nc.scalar.dma_start(out=wqk_sb[:, :, D:2 * D],
                            in_=wk_d.ap().rearrange("(t p) c -> p t c", p=128))
        nc.scalar.dma_start(out=wv_sb[:],
                            in_=wv_d.ap().rearrange("(t p) c -> p t c", p=128))
        wrk_sb = consts.tile([96, 2, D], FP16, tag="wrk")
        for u in range(2):
            nc.scalar.dma_start(out=wrk_sb[:, u, :], in_=wrk_d[ts(u, 96), :])
        pall = consts.tile([96, 2, S], FP16, tag="pall")
        nc.scalar.dma_start(out=pall[:, 0, :], in_=pseg_d[0:96, :])
        nc.scalar.dma_start(out=pall[:, 1, :], in_=pseg_d[96:192, :])
        widx_sb = persist.tile([128, WCOLS], U16, tag="widx")
        nc.scalar.dma_start(out=widx_sb[:], in_=widx_d.ap())

        # ---- persistent activations: per-chunk tiles for fine-grained deps --
        qcT_t = [persist.tile([D, 512], FP16, tag=f"qcT{c}", name=f"qcT{c}") for c in range(NC)]
        qpT_t = [persist.tile([D, 512], FP16, tag=f"qpT{c}", name=f"qpT{c}") for c in range(NC)]
        kT_t = [persist.tile([D, 512], FP16, tag=f"kT{c}", name=f"kT{c}") for c in range(NC)]
        rsT = persist.tile([D, S], FP16, tag="rsT")
        vext = persist.tile([128, NJ * (D + 1)], BF16, tag="vext")

        # rel-k table from the distinct positional columns
        ps_r = ppool_m.tile([128, S], F32, tag="ps_m")
        for u in range(2):
            nc.tensor.matmul(
                ps_r[0:D, :], wrk_sb[:, u, :], pall[:, u, :],
                start=(u == 0), stop=(u == 1),
            )
        nc.scalar.copy(out=rsT[:], in_=ps_r[0:D, :])

        xT_v = xT_d.ap().rearrange("(t p) n -> p t n", p=128)

        def proj_chunk(ic, xpool):
            x_sb = xpool.tile([128, KD, 512], FP16, tag="x")
            nc.sync.dma_start(out=x_sb[:], in_=xT_v[:, :, ds(ic * 512, 512)])
            ps_qk = ppool_ct.tile([128, 512], F32, tag="ps_ct")
            for kd in range(KD):
                nc.tensor.matmul(
                    ps_qk[:], wqk_sb[:, kd, :], x_sb[:, kd, :],
                    start=(kd == 0), stop=(kd == KD - 1),
                )
            nc.scalar.activation(
                out=qcT_t[ic][:], in_=ps_qk[0:D, :], func=AF.Identity,
                bias=bc_sb[:], scale=scale,
            )
            nc.scalar.activation(
                out=qpT_t[ic][:], in_=ps_qk[0:D, :], func=AF.Identity,
                bias=bp_sb[:], scale=scale,
            )
            nc.scalar.copy(out=kT_t[ic][:], in_=ps_qk[D:2 * D, :])
            for isb in range(4):
                J = ic * 4 + isb
                ps_v = ppool_op.tile([128, 512], F32, tag="ps_op")
                for kd in range(KD):
                    nc.tensor.matmul(
                        ps_v[:, 0:D], x_sb[:, kd, ts(isb, 128)], wv_sb[:, kd, :],
                        start=(kd == 0), stop=(kd == KD - 1),
                    )
                nc.scalar.copy(out=vext[:, ds(J * (D + 1), D)], in_=ps_v[:, 0:D])
                nc.vector.memset(vext[:, ds(J * (D + 1) + D, 1)], 1.0)

        shr_live = {}
        ec_live = {}
        out_pending = []

        def flush_out():
            while out_pending:
                o_tile, oi0 = out_pending.pop(0)
                nc.scalar.dma_start(out=out_d[ds(oi0, 128), :], in_=o_tile[:])

        def produce(g):
            # rel logits (distinct cols) + exp + GPSIMD window expansion +
            # em write + shear read, for both tiles of the pair
            for q in range(2):
                I = 2 * g + q
                i0 = I * 128
                ps_d = ppool_m.tile([128, S], F32, tag="ps_m")
                nc.tensor.matmul(
                    ps_d[:], qpT_t[i0 // 512][:, ds(i0 % 512, 128)], rsT[:],
                    start=True, stop=True,
                )
                u_sb = upool.tile([128, S], BF16, tag="u")
                nc.scalar.activation(out=u_sb[:], in_=ps_d[:], func=AF.Exp)
                em_sb = empool.tile([128, WN], BF16, tag="em")
                woff = (N - 128 - i0) // 16
                for c0 in range(0, WN, ic_chunk):
                    cw = min(ic_chunk, WN - c0)
                    nc.gpsimd.indirect_copy(
                        em_sb[:, ds(c0, cw)], u_sb[:],
                        widx_sb[:, ds(woff + c0 // 16, cw // 16)],
                        i_know_ap_gather_is_preferred=True,
                    )
                nc.sync.dma_start(out=em_d[ds(i0, 128), 0:WN - 1],
                                  in_=em_sb[:, 0:WN - 1])
            shr_pair = []
            for q in range(2):
                i0 = (2 * g + q) * 128
                shr_sb = shrpool.tile([128, N], BF16, tag="shr")
                shear_ap = bass.AP(em_d, i0 * WN + 127, [[WN - 1, 128], [1, N]])
                nc.sync.dma_start(out=shr_sb[:], in_=shear_ap)
                shr_pair.append(shr_sb)
            shr_live[g] = shr_pair

        def consume(g):
            i0g = g * 256
            shr_pair = shr_live.pop(g)
            flush_out()

            # content logits transposed: ecT[dj, J*256 + q*128 + di]
            ecT_sb = ecpool.tile([128, NJ * 256], BF16, tag="ecT")
            qc0, qc1 = qcT_t[i0g // 512], (i0g % 512) // 256
            for Jg in range(NJ // 2):
                ps = ppool_ct.tile([128, 512], F32, tag="ps_ct")
                for u in range(2):
                    J = Jg * 2 + u
                    nc.tensor.matmul(
                        ps[:, ts(u, 256)], kT_t[J // 4][:, ts(J % 4, 128)],
                        qc0[:, ts(qc1, 256)],
                        start=True, stop=True,
                    )
                nc.scalar.activation(
                    out=ecT_sb[:, ds(Jg * 512, 512)], in_=ps[:], func=AF.Exp,
                )

            # pT = ecT * shr^T
            pT_sb = work.tile([128, NJ * 256], BF16, tag="pT")
            for Jg in range(NJ // 4):
                ps_t = ppool_st.tile([128, 1024], BF16, tag="ps_st")
                for u in range(4):
                    J = Jg * 4 + u
                    for q in range(2):
                        nc.tensor.transpose(
                            ps_t[:, ds(u * 256 + q * 128, 128)],
                            shr_pair[q][:, ts(J, 128)], ident[:],
                        )
                nc.vector.tensor_mul(
                    pT_sb[:, ds(Jg * 1024, 1024)], ecT_sb[:, ds(Jg * 1024, 1024)], ps_t[:]
                )

            # PV + epilogue per q-tile
            for q in range(2):
                i0 = i0g + q * 128
                ps_o = ppool_epi.tile([128, 512], F32, tag="ps_o")
                for J in range(NJ):
                    nc.tensor.matmul(
                        ps_o[:, 0:D + 1], pT_sb[:, ds(J * 256 + q * 128, 128)],
                        vext[:, ds(J * (D + 1), D + 1)],
                        start=(J == 0), stop=(J == NJ - 1),
                    )
                rc_sb = sm.tile([128, 1], F32, tag="rc")
                nc.vector.reciprocal(out=rc_sb[:], in_=ps_o[:, D:D + 1])
                o_sb = sm.tile([128, D], BF16, tag="o")
                nc.vector.tensor_copy(o_sb[:], ps_o[:, 0:D])
                ps_ot = ps_o[0:D, 128:192].bitcast(BF16)
                nc.tensor.transpose(ps_ot, o_sb[:], ident[:])
                otT_sb = sm.tile([D, 128], BF16, tag="otT")
                nc.vector.tensor_copy(otT_sb[:], ps_ot)
                out_sb = work.tile([128, DIM], FP16, tag="out")
                for w in range(DIM // 512):
                    ps_op = ppool_op.tile([128, 512], F32, tag="ps_op")
                    nc.tensor.matmul(
                        ps_op[:], otT_sb[:], wo_sb[:, ts(w, 512)],
                        start=True, stop=True,
                    )
                    nc.vector.tensor_scalar_mul(
                        out_sb[:, ts(w, 512)], ps_op[:], rc_sb[:]
                    )
                out_pending.append((out_sb, i0))

        # ---- prep (x streamed through a transient pool), then the
        # software-pipelined produce/consume loop ----
        with tc.tile_pool(name="xpool", bufs=2) as xpool:
            for c in range(NC):
                proj_chunk(c, xpool)
        LAG = 1
        for g in range(Q // 2 + LAG):
            if g < Q // 2:
                produce(g)
            if g >= LAG:
                consume(g - LAG)
        flush_out()

    if split_waits:
        _patch_tile_drain()
        split_multi_waits(nc)
    return nc


# ---------------- host side ----------------

def get_positional_embed_np(seq_len, feature_size):
    distances = np.arange(-seq_len + 1, seq_len)
    nb = feature_size // 2
    pow_rate = math.exp(math.log(seq_len + 1) / nb)
    center_widths = np.power(np.float32(pow_rate), np.arange(1, nb + 1, dtype=np.float32)) - 1.0
    emb = (center_widths[None, :] > np.abs(distances)[:, None]).astype(np.float32)
    signed = np.sign(distances).astype(np.float32)[:, None] * emb
    return np.concatenate([emb, signed], axis=-1)  # [2n-1, F]


def make_in_maps(x, W_q, W_k, W_v, W_rel_k, W_out, rel_content_bias, rel_pos_bias):
    B, N, _ = np.asarray(x).shape
    WN = N + 128
    f16 = np.float16
    import ml_dtypes
    bf16 = ml_dtypes.bfloat16
    xT = np.ascontiguousarray(np.asarray(x[0], np.float32).T).astype(f16)
    pos = get_positional_embed_np(N, np.asarray(W_rel_k).shape[0])
    seg_starts, seg_of = pos_segments(N)
    S = len(seg_starts)
    pseg = np.ascontiguousarray(pos[seg_starts].T).astype(f16)  # [192, S]
    # wrapped master index table: window of q-tile I starts at rel index
    # N-128-128*I (multiple of 16), spans WN cols; idx list L = seg_of padded.
    L = np.zeros((2 * N - 1 + WN + 15) // 16 * 16, np.uint16)
    L[: 2 * N - 1] = seg_of.astype(np.uint16)
    WCOLS = len(L) // 16
    widx = np.zeros((128, WCOLS), np.uint16)
    Lw = L.reshape(WCOLS, 16).T  # [16, WCOLS]: Lw[w, s] = L[16s+w]
    for c in range(8):
        widx[16 * c:16 * (c + 1), :] = Lw
    in_maps = []
    for h in range(H):
        sl = slice(h * D, (h + 1) * D)
        in_maps.append({
            "xT": xT,
            "pseg": pseg,
            "widx": widx,
            "wq": np.ascontiguousarray(np.asarray(W_q)[:, sl]).astype(f16),
            "wk": np.ascontiguousarray(np.asarray(W_k)[:, sl]).astype(f16),
            "wv": np.ascontiguousarray(np.asarray(W_v)[:, sl]).astype(f16),
            "wrk": np.ascontiguousarray(np.asarray(W_rel_k)[:, sl]).astype(f16),
            "wo": np.ascontiguousarray(np.asarray(W_out)[sl, :]).astype(bf16),
            "bc": np.ascontiguousarray(
                np.asarray(rel_content_bias, np.float32)[0, h, 0, :].reshape(D, 1)),
            "bp": np.ascontiguousarray(
                np.asarray(rel_pos_bias, np.float32)[0, h, 0, :].reshape(D, 1)),
        })
    return in_maps


def combine_outputs(results, b_out):
    acc = None
    for r in results:
        p = r["out"].astype(np.float32)
        acc = p if acc is None else acc + p
    acc = acc + np.asarray(b_out, np.float32)[None, :]
    return acc[None]  # [1, N, DIM]


# ---------------- entry point ----------------

_NC_CACHE = {}


def kernel(x, W_q, W_k, W_v, W_rel_k, W_out, b_out,
           rel_content_bias, rel_pos_bias):
    """Full-input entry: shards per head across 8 NeuronCores, returns the
    full [1, N, 1536] float32 output."""
    from concourse import bass_utils

    x = np.asarray(x)
    N = x.shape[1]
    if N not in _NC_CACHE:
        _NC_CACHE[N] = build(N)
    nc = _NC_CACHE[N]
    in_maps = make_in_maps(x, W_q, W_k, W_v, W_rel_k, W_out,
                           rel_content_bias, rel_pos_bias)
    res = bass_utils.run_bass_kernel_spmd(nc, in_maps, core_ids=list(range(H)))
    return combine_outputs(res.results, b_out).astype(np.float32)



# revision 29
# speedup vs baseline: 1.0137x; 1.0137x over previous
"""Enformer-style relative-position attention (nn_Attention_27925877358942) for
8 Trainium2 NeuronCores.

Contract: kernel(**inputs) takes the FULL unsharded inputs (keys as in
setup_inputs()) and returns the full [1, 4096, 1536] float32 output.

Sharding: one head per core (8 heads / 8 cores). Host precomputes the
deterministic positional-feature table and x^T in fp16, slices per-head
weights, runs the SPMD Bass kernel via run_bass_kernel_spmd, and sums the
per-head output projections (+ b_out).

Device pipeline per core (head h), N=4096, d=64:
  - q^T,k^T (fp16, [64,N]) and v ([N,65] with ones col) projections on PE
  - r^T = (pos @ Wrelk_h)^T from the positional table
  - per query tile I: window logits em = exp((q+bp) . r[t0:t0+4223]) (ACT, bf16)
  - relative_shift via DRAM roundtrip: sheared strided read
      shr[di, j] = em[di, 127-di+j] (partition step = rowpitch-1 elements)
  - content logits transposed C^T = k_J . q_I (PE), exp on ACT
  - pT = exp(C^T) * transpose(shr) (PE transpose + DVE multiply, bf16)
  - O = pT.T @ [v|1] accumulated in PSUM; epilogue normalizes by the row sums
    and applies the per-head slice of W_out; host sums partials over heads.

This walrus build accepts at most ONE sync wait per instruction, so after
Tile scheduling every multi-wait instruction is split by inserting
wait-carrying NoOps just before it on the same engine (split_multi_waits),
and the Tile tail drain is built with the same constraint.
"""


_DRAIN_PATCHED = [False]


def _patch_tile_drain():
    if _DRAIN_PATCHED[0]:
        return
    _DRAIN_PATCHED[0] = True
    import concourse.tile as tile_mod
    from concourse.vector_clock import ScopedClock

    MAX_WAITS = 1

    def _drain_and_barrier(self, tick_clock, wait_clock):
        nc = self.nc
        drain_inst = nc.sync.drain()
        wait_clock.add_sem_waits(drain_inst.ins, ScopedClock({None: tick_clock.global_clock}))
        si = drain_inst.ins.sync_info
        waits = list(si.on_wait) if si is not None and si.on_wait else []
        if len(waits) > MAX_WAITS:
            si.on_wait = waits[:MAX_WAITS]
            rest = waits[MAX_WAITS:]
            import concourse.mybir as _mb
            for i in range(0, len(rest), MAX_WAITS):
                extra = nc.sync.drain()
                esi = extra.ins.sync_info
                if esi is None:
                    extra.ins.sync_info = _mb.SyncInfo(on_wait=rest[i:i + MAX_WAITS], on_update=[])
                else:
                    esi.on_wait = rest[i:i + MAX_WAITS]
        nc.all_engine_barrier()
        assert self.sems is not None
        popped = nc._tile_sem_poison_stack.pop()
        assert popped is self._sem_poison
        nc.clear_and_free_semaphores(list(self.sems.allocated().values()))
        nc.all_engine_barrier()

    tile_mod.TileContext._drain_and_barrier = _drain_and_barrier


def split_multi_waits(nc):
    """This walrus build allows at most ONE sync wait per instruction.
    Move extra waits onto InstNoOp carriers inserted just before, on the
    same engine queue (sequencers execute in order, so semantics hold)."""
    import concourse.mybir as mb
    n_split = 0
    for fn in nc.m.functions:
        for bb in fn.blocks:
            insts = list(bb.instructions)
            out = []
            for inst in insts:
                si = inst.sync_info
                waits = list(si.on_wait) if si is not None and si.on_wait else []
                if len(waits) > 1:
                    for w in waits[:-1]:
                        n_split += 1
                        nop = mb.InstNoOp(
                            name=f"waitsplit-{n_split}",
                            engine=inst.engine,
                            sync_info=mb.SyncInfo(on_wait=[w], on_update=[]),
                        )
                        out.append(nop)
                    si.on_wait = [waits[-1]]
                out.append(inst)
            if len(out) != len(insts):
                bb.instructions[:] = out
    return n_split


import math
from contextlib import ExitStack

import numpy as np

import concourse.bass as bass
import concourse.tile as tile
from concourse import mybir
from concourse.bass import ts, ds
from concourse.masks import make_identity

F32 = mybir.dt.float32
BF16 = mybir.dt.bfloat16
FP16 = mybir.dt.float16
U16 = mybir.dt.uint16
AF = mybir.ActivationFunctionType

DIM = 1536
H = 8
D = 64
NSEG = None  # set by build() from the static positional segmentation


def pos_segments(N):
    """Static segmentation of the 2N-1 relative positions: the central-mask
    features are piecewise constant in the distance, so the rel-k table has
    only ~157 distinct columns. Returns (seg_starts, seg_of)."""
    pos = get_positional_embed_np(N, 192)
    diffs = np.any(pos[1:] != pos[:-1], axis=1)
    seg_starts = np.concatenate([[0], np.nonzero(diffs)[0] + 1]).astype(np.int64)
    seg_of = np.zeros(2 * N - 1, np.int64)
    seg_of[seg_starts] = 1
    seg_of = np.cumsum(seg_of) - 1
    return seg_starts, seg_of


def build(N, split_waits=True, ic_chunk=1024, LAG=2, shr_bufs=6, em_bufs=3, ec_bufs=2, u_bufs=3):
    Q = N // 128           # query tiles
    NJ = N // 128          # key tiles
    WN = N + 128           # rel window width per q-tile (incl. 1 pad col)
    KD = DIM // 128        # contraction tiles for projections
    NC = N // 512          # projection chunks (one per xT DMA)
    S = len(pos_segments(N)[0])  # distinct rel-k columns (157 for N=4096)
    WCOLS = (2 * N - 1 + WN + 15) // 16  # wrapped master index cols

    nc = bass.Bass("TRN2", target_bir_lowering=False, debug=False)

    xT_d = nc.dram_tensor("xT", [DIM, N], FP16, kind="ExternalInput")
    pseg_d = nc.dram_tensor("pseg", [192, S], FP16, kind="ExternalInput")
    widx_d = nc.dram_tensor("widx", [128, WCOLS], U16, kind="ExternalInput")
    wq_d = nc.dram_tensor("wq", [DIM, D], FP16, kind="ExternalInput")
    wk_d = nc.dram_tensor("wk", [DIM, D], FP16, kind="ExternalInput")
    wv_d = nc.dram_tensor("wv", [DIM, D], FP16, kind="ExternalInput")
    wrk_d = nc.dram_tensor("wrk", [192, D], FP16, kind="ExternalInput")
    wo_d = nc.dram_tensor("wo", [D, DIM], BF16, kind="ExternalInput")
    bc_d = nc.dram_tensor("bc", [D, 1], F32, kind="ExternalInput")
    bp_d = nc.dram_tensor("bp", [D, 1], F32, kind="ExternalInput")
    out_d = nc.dram_tensor("out", [N, DIM], FP16, kind="ExternalOutput")
    em_d = nc.dram_tensor("em_scratch", [Q * 128, WN], BF16, kind="Internal")

    scale = D ** -0.5

    with tile.TileContext(nc) as tc, ExitStack() as ctx:
        consts = ctx.enter_context(tc.tile_pool(name="consts", bufs=1))
        persist = ctx.enter_context(tc.tile_pool(name="persist", bufs=1))
        work = ctx.enter_context(tc.tile_pool(name="work", bufs=2))
        ecpool = ctx.enter_context(tc.tile_pool(name="ecpool", bufs=ec_bufs))
        upool = ctx.enter_context(tc.tile_pool(name="upool", bufs=u_bufs))
        empool = ctx.enter_context(tc.tile_pool(name="empool", bufs=em_bufs))
        shrpool = ctx.enter_context(tc.tile_pool(name="shrpool", bufs=shr_bufs))
        sm = ctx.enter_context(tc.tile_pool(name="sm", bufs=2))
        ppool_m = ctx.enter_context(tc.tile_pool(name="ppool_m", bufs=1, space="PSUM"))
        ppool_ct = ctx.enter_context(tc.tile_pool(name="ppool_ct", bufs=2, space="PSUM"))
        ppool_st = ctx.enter_context(tc.tile_pool(name="ppool_st", bufs=2, space="PSUM"))
        ppool_epi = ctx.enter_context(tc.tile_pool(name="ppool_epi", bufs=1, space="PSUM"))
        ppool_op = ctx.enter_context(tc.tile_pool(name="ppool_op", bufs=2, space="PSUM"))

        # ---- constants (small loads via ACT queue; SP stays for x/em/shear) --
        ident = consts.tile([128, 128], BF16, tag="ident")
        make_identity(nc, ident[:])
        bc_sb = consts.tile([D, 1], F32, tag="bc")
        nc.scalar.dma_start(out=bc_sb[:], in_=bc_d.ap())
        bp_sb = consts.tile([D, 1], F32, tag="bp")
        nc.scalar.dma_start(out=bp_sb[:], in_=bp_d.ap())
        wo_sb = consts.tile([D, DIM], BF16, tag="wo")
        nc.scalar.dma_start(out=wo_sb[:], in_=wo_d.ap())
        wqk_sb = consts.tile([128, KD, 2 * D], FP16, tag="wqk")
        wv_sb = consts.tile([128, KD, D], FP16, tag="wv")
        nc.scalar.dma_start(out=wqk_sb[:, :, 0:D],
                            in_=wq_d.ap().rearrange("(t p) c -> p t c", p=128))
        nc.scalar.dma_start(out=wqk_sb[:, :, D:2 * D],
                            in_=wk_d.ap().rearrange("(t p) c -> p t c", p=128))
        nc.scalar.dma_start(out=wv_sb[:],
                            in_=wv_d.ap().rearrange("(t p) c -> p t c", p=128))
        wrk_sb = consts.tile([96, 2, D], FP16, tag="wrk")
        for u in range(2):
            nc.scalar.dma_start(out=wrk_sb[:, u, :], in_=wrk_d[ts(u, 96), :])
        pall = consts.tile([96, 2, S], FP16, tag="pall")
        nc.scalar.dma_start(out=pall[:, 0, :], in_=pseg_d[0:96, :])
        nc.scalar.dma_start(out=pall[:, 1, :], in_=pseg_d[96:192, :])
        widx_sb = persist.tile([128, WCOLS], U16, tag="widx")
        nc.scalar.dma_start(out=widx_sb[:], in_=widx_d.ap())

        # ---- persistent activations: per-chunk tiles for fine-grained deps --
        qcT_t = [persist.tile([D, 512], FP16, tag=f"qcT{c}", name=f"qcT{c}") for c in range(NC)]
        qpT_t = [persist.tile([D, 512], FP16, tag=f"qpT{c}", name=f"qpT{c}") for c in range(NC)]
        kT_t = [persist.tile([D, 512], FP16, tag=f"kT{c}", name=f"kT{c}") for c in range(NC)]
        rsT = persist.tile([D, S], FP16, tag="rsT")
        vext = persist.tile([128, NJ * (D + 1)], BF16, tag="vext")

        # rel-k table from the distinct positional columns
        ps_r = ppool_m.tile([128, S], F32, tag="ps_m")
        for u in range(2):
            nc.tensor.matmul(
                ps_r[0:D, :], wrk_sb[:, u, :], pall[:, u, :],
                start=(u == 0), stop=(u == 1),
            )
        nc.scalar.copy(out=rsT[:], in_=ps_r[0:D, :])

        xT_v = xT_d.ap().rearrange("(t p) n -> p t n", p=128)

        def proj_chunk(ic, xpool):
            x_sb = xpool.tile([128, KD, 512], FP16, tag="x")
            nc.sync.dma_start(out=x_sb[:], in_=xT_v[:, :, ds(ic * 512, 512)])
            ps_qk = ppool_ct.tile([128, 512], F32, tag="ps_ct")
            for kd in range(KD):
                nc.tensor.matmul(
                    ps_qk[:], wqk_sb[:, kd, :], x_sb[:, kd, :],
                    start=(kd == 0), stop=(kd == KD - 1),
                )
            nc.scalar.activation(
                out=qcT_t[ic][:], in_=ps_qk[0:D, :], func=AF.Identity,
                bias=bc_sb[:], scale=scale,
            )
            nc.scalar.activation(
                out=qpT_t[ic][:], in_=ps_qk[0:D, :], func=AF.Identity,
                bias=bp_sb[:], scale=scale,
            )
            nc.scalar.copy(out=kT_t[ic][:], in_=ps_qk[D:2 * D, :])
            for isb in range(4):
                J = ic * 4 + isb
                ps_v = ppool_op.tile([128, 512], F32, tag="ps_op")
                for kd in range(KD):
                    nc.tensor.matmul(
                        ps_v[:, 0:D], x_sb[:, kd, ts(isb, 128)], wv_sb[:, kd, :],
                        start=(kd == 0), stop=(kd == KD - 1),
                    )
                nc.scalar.copy(out=vext[:, ds(J * (D + 1), D)], in_=ps_v[:, 0:D])
                nc.vector.memset(vext[:, ds(J * (D + 1) + D, 1)], 1.0)

        shr_live = {}
        ec_live = {}
        out_pending = []

        def flush_out():
            while out_pending:
                o_tile, oi0 = out_pending.pop(0)
                nc.scalar.dma_start(out=out_d[ds(oi0, 128), :], in_=o_tile[:])

        def produce(g):
            # rel logits (distinct cols) + exp + GPSIMD window expansion +
            # em write + shear read, for both tiles of the pair
            for q in range(2):
                I = 2 * g + q
                i0 = I * 128
                ps_d = ppool_m.tile([128, S], F32, tag="ps_m")
                nc.tensor.matmul(
                    ps_d[:], qpT_t[i0 // 512][:, ds(i0 % 512, 128)], rsT[:],
                    start=True, stop=True,
                )
                u_sb = upool.tile([128, S], BF16, tag="u")
                nc.scalar.activation(out=u_sb[:], in_=ps_d[:], func=AF.Exp)
                em_sb = empool.tile([128, WN], BF16, tag="em")
                woff = (N - 128 - i0) // 16
                for c0 in range(0, WN, ic_chunk):
                    cw = min(ic_chunk, WN - c0)
                    nc.gpsimd.indirect_copy(
                        em_sb[:, ds(c0, cw)], u_sb[:],
                        widx_sb[:, ds(woff + c0 // 16, cw // 16)],
                        i_know_ap_gather_is_preferred=True,
                    )
                nc.sync.dma_start(out=em_d[ds(i0, 128), 0:WN - 1],
                                  in_=em_sb[:, 0:WN - 1])
            shr_pair = []
            for q in range(2):
                i0 = (2 * g + q) * 128
                shr_sb = shrpool.tile([128, N], BF16, tag="shr")
                shear_ap = bass.AP(em_d, i0 * WN + 127, [[WN - 1, 128], [1, N]])
                nc.sync.dma_start(out=shr_sb[:], in_=shear_ap)
                shr_pair.append(shr_sb)
            shr_live[g] = shr_pair

        def consume(g):
            i0g = g * 256
            shr_pair = shr_live.pop(g)
            flush_out()

            # content logits transposed: ecT[dj, J*256 + q*128 + di]
            ecT_sb = ecpool.tile([128, NJ * 256], BF16, tag="ecT")
            qc0, qc1 = qcT_t[i0g // 512], (i0g % 512) // 256
            for Jg in range(NJ // 2):
                ps = ppool_ct.tile([128, 512], F32, tag="ps_ct")
                for u in range(2):
                    J = Jg * 2 + u
                    nc.tensor.matmul(
                        ps[:, ts(u, 256)], kT_t[J // 4][:, ts(J % 4, 128)],
                        qc0[:, ts(qc1, 256)],
                        start=True, stop=True,
                    )
                nc.scalar.activation(
                    out=ecT_sb[:, ds(Jg * 512, 512)], in_=ps[:], func=AF.Exp,
                )

            # pT = ecT * shr^T
            pT_sb = work.tile([128, NJ * 256], BF16, tag="pT")
            for Jg in range(NJ // 4):
                ps_t = ppool_st.tile([128, 1024], BF16, tag="ps_st")
                for u in range(4):
                    J = Jg * 4 + u
                    for q in range(2):
                        nc.tensor.transpose(
                            ps_t[:, ds(u * 256 + q * 128, 128)],
                            shr_pair[q][:, ts(J, 128)], ident[:],
                        )
                nc.vector.tensor_mul(
                    pT_sb[:, ds(Jg * 1024, 1024)], ecT_sb[:, ds(Jg * 1024, 1024)], ps_t[:]
                )

            # PV + epilogue per q-tile
            for q in range(2):
                i0 = i0g + q * 128
                ps_o = ppool_epi.tile([128, 512], F32, tag="ps_o")
                for J in range(NJ):
                    nc.tensor.matmul(
                        ps_o[:, 0:D + 1], pT_sb[:, ds(J * 256 + q * 128, 128)],
                        vext[:, ds(J * (D + 1), D + 1)],
                        start=(J == 0), stop=(J == NJ - 1),
                    )
                rc_sb = sm.tile([128, 1], F32, tag="rc")
                nc.vector.reciprocal(out=rc_sb[:], in_=ps_o[:, D:D + 1])
                o_sb = sm.tile([128, D], BF16, tag="o")
                nc.vector.tensor_copy(o_sb[:], ps_o[:, 0:D])
                ps_ot = ps_o[0:D, 128:192].bitcast(BF16)
                nc.tensor.transpose(ps_ot, o_sb[:], ident[:])
                otT_sb = sm.tile([D, 128], BF16, tag="otT")
                nc.vector.tensor_copy(otT_sb[:], ps_ot)
                out_sb = work.tile([128, DIM], FP16, tag="out")
                for w in range(DIM // 512):
                    ps_op = ppool_op.tile([128, 512], F32, tag="ps_op")
                    nc.tensor.matmul(
                        ps_op[:], otT_sb[:], wo_sb[:, ts(w, 512)],
                        start=True, stop=True,
                    )
                    nc.vector.tensor_scalar_mul(
                        out_sb[:, ts(w, 512)], ps_op[:], rc_sb[:]
                    )
                out_pending.append((out_sb, i0))

        # ---- prep (x streamed through a transient pool), then the
        # software-pipelined produce/consume loop ----
        with tc.tile_pool(name="xpool", bufs=2) as xpool:
            for c in range(NC):
                proj_chunk(c, xpool)
        for g in range(Q // 2 + LAG):
            if g < Q // 2:
                produce(g)
            if g >= LAG:
                consume(g - LAG)
        flush_out()

    if split_waits:
        _patch_tile_drain()
        split_multi_waits(nc)
    return nc


# ---------------- host side ----------------

def get_positional_embed_np(seq_len, feature_size):
    distances = np.arange(-seq_len + 1, seq_len)
    nb = feature_size // 2
    pow_rate = math.exp(math.log(seq_len + 1) / nb)
    center_widths = np.power(np.float32(pow_rate), np.arange(1, nb + 1, dtype=np.float32)) - 1.0
    emb = (center_widths[None, :] > np.abs(distances)[:, None]).astype(np.float32)
    signed = np.sign(distances).astype(np.float32)[:, None] * emb
    return np.concatenate([emb, signed], axis=-1)  # [2n-1, F]


def make_in_maps(x, W_q, W_k, W_v, W_rel_k, W_out, rel_content_bias, rel_pos_bias):
    B, N, _ = np.asarray(x).shape
    WN = N + 128
    f16 = np.float16
    import ml_dtypes
    bf16 = ml_dtypes.bfloat16
    xT = np.ascontiguousarray(np.asarray(x[0], np.float32).T).astype(f16)
    pos = get_positional_embed_np(N, np.asarray(W_rel_k).shape[0])
    seg_starts, seg_of = pos_segments(N)
    S = len(seg_starts)
    pseg = np.ascontiguousarray(pos[seg_starts].T).astype(f16)  # [192, S]
    # wrapped master index table: window of q-tile I starts at rel index
    # N-128-128*I (multiple of 16), spans WN cols; idx list L = seg_of padded.
    L = np.zeros((2 * N - 1 + WN + 15) // 16 * 16, np.uint16)
    L[: 2 * N - 1] = seg_of.astype(np.uint16)
    WCOLS = len(L) // 16
    widx = np.zeros((128, WCOLS), np.uint16)
    Lw = L.reshape(WCOLS, 16).T  # [16, WCOLS]: Lw[w, s] = L[16s+w]
    for c in range(8):
        widx[16 * c:16 * (c + 1), :] = Lw
    in_maps = []
    for h in range(H):
        sl = slice(h * D, (h + 1) * D)
        in_maps.append({
            "xT": xT,
            "pseg": pseg,
            "widx": widx,
            "wq": np.ascontiguousarray(np.asarray(W_q)[:, sl]).astype(f16),
            "wk": np.ascontiguousarray(np.asarray(W_k)[:, sl]).astype(f16),
            "wv": np.ascontiguousarray(np.asarray(W_v)[:, sl]).astype(f16),
            "wrk": np.ascontiguousarray(np.asarray(W_rel_k)[:, sl]).astype(f16),
            "wo": np.ascontiguousarray(np.asarray(W_out)[sl, :]).astype(bf16),
            "bc": np.ascontiguousarray(
                np.asarray(rel_content_bias, np.float32)[0, h, 0, :].reshape(D, 1)),
            "bp": np.ascontiguousarray(
                np.asarray(rel_pos_bias, np.float32)[0, h, 0, :].reshape(D, 1)),
        })
    return in_maps


def combine_outputs(results, b_out):
    acc = None
    for r in results:
        p = r["out"].astype(np.float32)
        acc = p if acc is None else acc + p
    acc = acc + np.asarray(b_out, np.float32)[None, :]
    return acc[None]  # [1, N, DIM]


# ---------------- entry point ----------------

_NC_CACHE = {}


def kernel(x, W_q, W_k, W_v, W_rel_k, W_out, b_out,
           rel_content_bias, rel_pos_bias):
    """Full-input entry: shards per head across 8 NeuronCores, returns the
    full [1, N, 1536] float32 output."""
    from concourse import bass_utils

    x = np.asarray(x)
    N = x.shape[1]
    if N not in _NC_CACHE:
        _NC_CACHE[N] = build(N)
    nc = _NC_CACHE[N]
    in_maps = make_in_maps(x, W_q, W_k, W_v, W_rel_k, W_out,
                           rel_content_bias, rel_pos_bias)
    res = bass_utils.run_bass_kernel_spmd(nc, in_maps, core_ids=list(range(H)))
    return combine_outputs(res.results, b_out).astype(np.float32)

